# revision 17
# baseline (speedup 1.0000x reference)
"""FFM cell (complex gated multiply-add) kernel for Trainium2, 8 NeuronCores.

Computes, for t in [0,T), r in [0,TRACE), c in [0,CTX):
    decay[t,r] = exp(-|a[r]| * t)        (t = float(j[t]))
    g_re[t,r,c] = decay[t,r] * cos(b[c]*t)
    g_im[t,r,c] = decay[t,r] * sin(b[c]*t)
    new_re = state_re*g_re - state_im*g_im + x_re
    new_im = state_re*g_im + state_im*g_re + x_im
Returns (stack([new_re,new_im], -1), j + i).

Strategy: shard T across the 8 cores (pure data parallel). The small
per-axis factor tables decay[T,TRACE], cos[T,CTX], sin[T,CTX] are
precomputed on host in f32 (exact match to the reference math, avoids
on-device trig range-reduction issues) and streamed in as side inputs
(3 MiB total vs 384 MiB of main traffic). On device, each [128t x FD]
tile does 10 f32 elementwise tensor-tensor ops; the rank-1 factors are
read through stride-0 (broadcast) access patterns so they are never
materialized at full rank.
"""

import numpy as np

import concourse.bass as bass
import concourse.mybir as mybir
import concourse.tile as tile
from concourse.bass_utils import run_bass_kernel_spmd

T, TRACE, CTX = 4096, 64, 64
N_CORES = 8
T_LOC = T // N_CORES            # 512 t-rows per core
F = TRACE * CTX                 # 4096 features per t
P = 128                         # SBUF partitions (t per tile)
RB = 32                         # trace-block per iteration
FD = RB * CTX                   # 2048 free elements per iteration
N_TTILE = T_LOC // P            # 4 t-tiles per core
N_RB = TRACE // RB              # 2 r-blocks per t-tile

F32 = mybir.dt.float32
I32 = mybir.dt.int32


MAX_WAITS_PER_INST = 1


def cap_sync_waits(nc: bass.Bass, limit: int = MAX_WAITS_PER_INST):
    """walrus's CoreV3 codegen rejects instructions carrying more than a
    couple of sync waits ("Too many sync wait commands"). Hoist extra waits
    onto single-wait NOPs inserted just before, on the same engine."""
    for bb in nc.main_func.blocks:
        insts = bb.instructions
        idx = 0
        while idx < len(insts):
            ins = insts[idx]
            si = ins.sync_info
            if si is not None and si.on_wait and len(si.on_wait) > limit:
                waits = list(si.on_wait)
                si.on_wait = waits[:limit]
                extra = waits[limit:]
                for w in extra:
                    nop = mybir.InstNoOp(
                        name=nc.get_next_instruction_name(),
                        engine=ins.engine,
                        sync_info=mybir.SyncInfo(on_wait=[w], on_update=[]),
                        bass_nofuse=True,
                    )
                    insts.insert(idx, nop)
                    idx += 1
            idx += 1


class SplitDrainTileContext(tile.TileContext):
    """Tile's kernel-tail drain piles every outstanding sem wait onto one
    Drain instruction; walrus's TPB_CTRL codegen rejects >1-2 sync waits
    per instruction ("Too many sync wait commands"). Spread the waits over
    standalone single-wait NOPs instead."""

    def _drain_and_barrier(self, tick_clock, wait_clock):
        nc = self.nc
        probe = nc.sync.drain()
        wait_clock.add_sem_waits(
            probe.ins, tile.ScopedClock({None: tick_clock.global_clock})
        )
        si = probe.ins.sync_info
        waits = list(si.on_wait or []) if si else []
        if si and si.on_wait and len(waits) > 1:
            si.on_wait = [waits[0]]
            for w in waits[1:]:
                nop = nc.sync.nop(nofuse=True)
                nsi = nop.ins.sync_info
                if nsi is None:
                    nop.ins.sync_info = mybir.SyncInfo(on_wait=[w], on_update=[])
                else:
                    nsi.on_wait = (nsi.on_wait or []) + [w]
        nc.all_engine_barrier()
        assert self.sems is not None
        popped = nc._tile_sem_poison_stack.pop()
        assert popped is self._sem_poison
        nc.clear_and_free_semaphores(list(self.sems.allocated().values()))
        nc.all_engine_barrier()


def build_program(cap_waits: bool = True) -> bass.Bass:
    nc = bass.Bass()

    sr_d = nc.declare_dram_parameter("state_re", [T_LOC, F], F32, isOutput=False)
    si_d = nc.declare_dram_parameter("state_im", [T_LOC, F], F32, isOutput=False)
    xr_d = nc.declare_dram_parameter("x_re", [T_LOC, F], F32, isOutput=False)
    xi_d = nc.declare_dram_parameter("x_im", [T_LOC, F], F32, isOutput=False)
    dec_d = nc.declare_dram_parameter("dec", [T_LOC, TRACE], F32, isOutput=False)
    cos_d = nc.declare_dram_parameter("cosv", [T_LOC, CTX], F32, isOutput=False)
    sin_d = nc.declare_dram_parameter("sinv", [T_LOC, CTX], F32, isOutput=False)
    ji_d = nc.declare_dram_parameter("jv", [T_LOC], I32, isOutput=False)
    ii_d = nc.declare_dram_parameter("iv", [T_LOC], I32, isOutput=False)

    out_d = nc.declare_dram_parameter("out", [T_LOC, 2 * F], F32, isOutput=True)
    cnt_d = nc.declare_dram_parameter("cnt", [T_LOC], I32, isOutput=True)

    with SplitDrainTileContext(nc) as tc:
        with (
            tc.tile_pool(name="inp", bufs=2) as inp,
            tc.tile_pool(name="tmp", bufs=2) as tmp,
            tc.tile_pool(name="outp", bufs=2) as outp,
            tc.tile_pool(name="small", bufs=2) as small,
        ):
            # counter output: cnt = j + i (int32), trivial
            jt = small.tile([P, T_LOC // P], I32, tag="jt")
            it = small.tile([P, T_LOC // P], I32, tag="it")
            ct = small.tile([P, T_LOC // P], I32, tag="ct")
            nc.sync.dma_start(jt[:], ji_d[:].rearrange("(p f) -> p f", p=P))
            nc.sync.dma_start(it[:], ii_d[:].rearrange("(p f) -> p f", p=P))
            nc.gpsimd.tensor_add(ct[:], jt[:], it[:])
            nc.sync.dma_start(cnt_d[:].rearrange("(p f) -> p f", p=P), ct[:])

            for tt in range(N_TTILE):
                t0 = tt * P
                # per-t-tile factor tables [128, 64]
                dct = small.tile([P, TRACE], F32, tag="dct")
                cst = small.tile([P, CTX], F32, tag="cst")
                snt = small.tile([P, CTX], F32, tag="snt")
                nc.sync.dma_start(dct[:], dec_d[t0:t0 + P, :])
                nc.sync.dma_start(cst[:], cos_d[t0:t0 + P, :])
                nc.sync.dma_start(snt[:], sin_d[t0:t0 + P, :])

                for rb in range(N_RB):
                    f0 = rb * FD
                    sr = inp.tile([P, FD], F32, tag="sr")
                    si = inp.tile([P, FD], F32, tag="si")
                    xr = inp.tile([P, FD], F32, tag="xr")
                    xi = inp.tile([P, FD], F32, tag="xi")
                    nc.sync.dma_start(sr[:], sr_d[t0:t0 + P, f0:f0 + FD])
                    nc.sync.dma_start(si[:], si_d[t0:t0 + P, f0:f0 + FD])
                    nc.sync.dma_start(xr[:], xr_d[t0:t0 + P, f0:f0 + FD])
                    nc.sync.dma_start(xi[:], xi_d[t0:t0 + P, f0:f0 + FD])

                    gre = tmp.tile([P, FD], F32, tag="gre")
                    gim = tmp.tile([P, FD], F32, tag="gim")
                    p1 = tmp.tile([P, FD], F32, tag="p1")
                    ot = outp.tile([P, 2 * FD], F32, tag="ot")

                    # 3D views [P, RB, CTX]
                    sr3 = sr[:].rearrange("p (r c) -> p r c", c=CTX)
                    si3 = si[:].rearrange("p (r c) -> p r c", c=CTX)
                    xr3 = xr[:].rearrange("p (r c) -> p r c", c=CTX)
                    xi3 = xi[:].rearrange("p (r c) -> p r c", c=CTX)
                    gre3 = gre[:].rearrange("p (r c) -> p r c", c=CTX)
                    gim3 = gim[:].rearrange("p (r c) -> p r c", c=CTX)
                    p13 = p1[:].rearrange("p (r c) -> p r c", c=CTX)
                    cos_b = cst[:].unsqueeze(1).broadcast_to([P, RB, CTX])
                    sin_b = snt[:].unsqueeze(1).broadcast_to([P, RB, CTX])
                    dec_b = (
                        dct[:, rb * RB:(rb + 1) * RB]
                        .unsqueeze(2)
                        .broadcast_to([P, RB, CTX])
                    )
                    # strided views of the interleaved out tile
                    o_re = ot[:, 0:2 * FD:2]
                    o_im = ot[:, 1:2 * FD:2]

                    # Same association as the reference:
                    #   g_re = dec*cos; g_im = dec*sin
                    #   new_re = (sr*g_re - si*g_im) + xr
                    #   new_im = (sr*g_im + si*g_re) + xi
                    # TensorTensor runs on DVE or Pool only; all TT writes are
                    # unit-stride. The (re,im)->stride-2 interleave runs as two
                    # 1-input copies on the otherwise-idle ACT engine.
                    nc.vector.tensor_mul(gre3, dec_b, cos_b)
                    nc.gpsimd.tensor_mul(gim3, dec_b, sin_b)
                    nc.vector.tensor_mul(p13, sr3, gre3)
                    nc.vector.tensor_mul(sr3, sr3, gim3)   # q1
                    nc.gpsimd.tensor_mul(gim3, si3, gim3)  # p2
                    nc.vector.tensor_mul(si3, si3, gre3)   # q2
                    nc.vector.tensor_sub(p13, p13, gim3)   # w = p1 - p2
                    nc.gpsimd.tensor_add(sr3, sr3, si3)    # z = q1 + q2
                    nc.vector.tensor_add(p13, p13, xr3)    # new_re
                    nc.vector.tensor_add(sr3, sr3, xi3)    # new_im
                    nc.scalar.copy(o_re, p1[:])
                    nc.scalar.copy(o_im, sr[:])

                    nc.sync.dma_start(
                        out_d[t0:t0 + P, 2 * f0:2 * f0 + 2 * FD], ot[:]
                    )
    if cap_waits:
        cap_sync_waits(nc)
    return nc


_NC_CACHE: list[bass.Bass | None] = [None]


def _factor_tables(a, b, j):
    """decay/cos/sin tables [T, 64] computed with jnp on the default jax
    backend, mirroring the reference's expressions exactly (same ops, same
    shapes) so the table values match the reference bit-for-bit on
    whichever platform the reference runs."""
    import jax.numpy as jnp
    aj, bj, jj = jnp.asarray(a), jnp.asarray(b), jnp.asarray(j)
    t = jj.astype(jnp.float32)[:, None, None]
    decay = jnp.exp(-jnp.abs(aj)[None, :, None] * t)    # [T,TRACE,1]
    theta = bj[None, None, :] * t                       # [T,1,CTX]
    cosv = jnp.cos(theta)
    sinv = jnp.sin(theta)
    return (np.asarray(decay[:, :, 0]), np.asarray(cosv[:, 0, :]),
            np.asarray(sinv[:, 0, :]))


def make_in_maps(inputs):
    state_re, state_im = inputs["state_re"], inputs["state_im"]
    x_re, x_im = inputs["x_re"], inputs["x_im"]
    a, b, i, j = inputs["a"], inputs["b"], inputs["i"], inputs["j"]
    dec, cosv, sinv = _factor_tables(a, b, j)
    assert dec.dtype == np.float32 and cosv.dtype == np.float32

    sr = np.ascontiguousarray(state_re.reshape(T, F))
    si = np.ascontiguousarray(state_im.reshape(T, F))
    xr = np.ascontiguousarray(x_re.reshape(T, F))
    xi = np.ascontiguousarray(x_im.reshape(T, F))

    in_maps = []
    for k in range(N_CORES):
        s = slice(k * T_LOC, (k + 1) * T_LOC)
        in_maps.append({
            "state_re": sr[s], "state_im": si[s],
            "x_re": xr[s], "x_im": xi[s],
            "dec": np.ascontiguousarray(dec[s]),
            "cosv": np.ascontiguousarray(cosv[s]),
            "sinv": np.ascontiguousarray(sinv[s]),
            "jv": np.ascontiguousarray(j[s]),
            "iv": np.ascontiguousarray(i[s]),
        })
    return in_maps


def kernel(state_re, state_im, x_re, x_im, a, b, i, j):
    in_maps = make_in_maps(dict(
        state_re=state_re, state_im=state_im, x_re=x_re, x_im=x_im,
        a=a, b=b, i=i, j=j,
    ))

    if _NC_CACHE[0] is None:
        _NC_CACHE[0] = build_program()
    nc = _NC_CACHE[0]

    res = run_bass_kernel_spmd(nc, in_maps, core_ids=list(range(N_CORES)))
    out = np.concatenate(
        [r["out"].reshape(T_LOC, TRACE, CTX, 2) for r in res.results], axis=0
    )
    cnt = np.concatenate([r["cnt"] for r in res.results], axis=0).astype(np.int32)
    return out, cnt


# revision 18
# speedup vs baseline: 1.1677x; 1.1677x over previous
"""FFM cell (complex gated multiply-add) kernel for Trainium2, 8 NeuronCores.

Computes, for t in [0,T), r in [0,TRACE), c in [0,CTX):
    decay[t,r] = exp(-|a[r]| * t)        (t = float(j[t]))
    g_re[t,r,c] = decay[t,r] * cos(b[c]*t)
    g_im[t,r,c] = decay[t,r] * sin(b[c]*t)
    new_re = state_re*g_re - state_im*g_im + x_re
    new_im = state_re*g_im + state_im*g_re + x_im
Returns (stack([new_re,new_im], -1), j + i).

Strategy: shard T across the 8 cores (pure data parallel). Host precomputes
the small per-axis tables decay[T,64], cos[T,64], sin[T,64] with jnp on the
default jax backend (mirrors the reference's own table math exactly), plus
two interleaved trig tables csn1=(cos,sin) and csn2=(-sin,cos), and
interleaves x into x_int[T,r,c,2] (pure layout, no arithmetic).

On device, per [128t x 32r] tile:
  - ACT prescales state by decay in place: 64 small activation(Copy,
    scale=dec[:,r]) ops (per-partition scale = exact fp32 FMA). ACT has its
    own SBUF port, so this is truly additive capacity.
  - DVE does only 4 unit-stride f32 TT ops of 4096: A = srd (dup'd via a
    stride-0 4D AP) * csn1, B = sid * csn2, AB = A+B, out = AB + x_int.
    The output lands directly in (re,im)-interleaved layout - no strided
    writes, no GpSimd (whose SBUF port is shared with DVE and measurably
    degrades it).
DMA: 48.4 MiB per core total at ~358 GB/s -> ~141 us/core roofline.
"""

import numpy as np

import concourse.bass as bass
import concourse.mybir as mybir
import concourse.tile as tile
from concourse.bass_utils import run_bass_kernel_spmd

T, TRACE, CTX = 4096, 64, 64
N_CORES = 8
T_LOC = T // N_CORES            # 512 t-rows per core
F = TRACE * CTX                 # 4096 features per t
P = 128                         # SBUF partitions (t per tile)
RB = 32                         # trace-block per iteration
FD = RB * CTX                   # 2048 state elements per iteration
N_TTILE = T_LOC // P            # 4 t-tiles per core
N_RB = TRACE // RB              # 2 r-blocks per t-tile

F32 = mybir.dt.float32
I32 = mybir.dt.int32

MAX_WAITS_PER_INST = 1


def cap_sync_waits(nc: bass.Bass, limit: int = MAX_WAITS_PER_INST):
    """walrus's CoreV3 codegen rejects instructions carrying more than one
    sync wait ("Too many sync wait commands"). Hoist extra waits onto
    single-wait NOPs inserted just before, on the same engine."""
    for bb in nc.main_func.blocks:
        insts = bb.instructions
        idx = 0
        while idx < len(insts):
            ins = insts[idx]
            si = ins.sync_info
            if si is not None and si.on_wait and len(si.on_wait) > limit:
                waits = list(si.on_wait)
                si.on_wait = waits[:limit]
                extra = waits[limit:]
                for w in extra:
                    nop = mybir.InstNoOp(
                        name=nc.get_next_instruction_name(),
                        engine=ins.engine,
                        sync_info=mybir.SyncInfo(on_wait=[w], on_update=[]),
                        bass_nofuse=True,
                    )
                    insts.insert(idx, nop)
                    idx += 1
            idx += 1


class SplitDrainTileContext(tile.TileContext):
    """Tile's kernel-tail drain piles every outstanding sem wait onto one
    Drain instruction; walrus rejects >1 sync waits per instruction. Spread
    the waits over standalone single-wait NOPs instead."""

    def _drain_and_barrier(self, tick_clock, wait_clock):
        nc = self.nc
        probe = nc.sync.drain()
        wait_clock.add_sem_waits(
            probe.ins, tile.ScopedClock({None: tick_clock.global_clock})
        )
        si = probe.ins.sync_info
        waits = list(si.on_wait or []) if si else []
        if si and si.on_wait and len(waits) > 1:
            si.on_wait = [waits[0]]
            for w in waits[1:]:
                nop = nc.sync.nop(nofuse=True)
                nsi = nop.ins.sync_info
                if nsi is None:
                    nop.ins.sync_info = mybir.SyncInfo(on_wait=[w], on_update=[])
                else:
                    nsi.on_wait = (nsi.on_wait or []) + [w]
        nc.all_engine_barrier()
        assert self.sems is not None
        popped = nc._tile_sem_poison_stack.pop()
        assert popped is self._sem_poison
        nc.clear_and_free_semaphores(list(self.sems.allocated().values()))
        nc.all_engine_barrier()


def build_program(cap_waits: bool = True) -> bass.Bass:
    nc = bass.Bass()

    sr_d = nc.declare_dram_parameter("state_re", [T_LOC, F], F32, isOutput=False)
    si_d = nc.declare_dram_parameter("state_im", [T_LOC, F], F32, isOutput=False)
    x_d = nc.declare_dram_parameter("x_int", [T_LOC, 2 * F], F32, isOutput=False)
    dec_d = nc.declare_dram_parameter("dec", [T_LOC, TRACE], F32, isOutput=False)
    c1_d = nc.declare_dram_parameter("csn1", [T_LOC, 2 * CTX], F32, isOutput=False)
    c2_d = nc.declare_dram_parameter("csn2", [T_LOC, 2 * CTX], F32, isOutput=False)
    ji_d = nc.declare_dram_parameter("jv", [T_LOC], I32, isOutput=False)
    ii_d = nc.declare_dram_parameter("iv", [T_LOC], I32, isOutput=False)

    out_d = nc.declare_dram_parameter("out", [T_LOC, 2 * F], F32, isOutput=True)
    cnt_d = nc.declare_dram_parameter("cnt", [T_LOC], I32, isOutput=True)

    CP = mybir.ActivationFunctionType.Copy

    with SplitDrainTileContext(nc) as tc:
        with (
            tc.tile_pool(name="inp", bufs=3) as inp,
            tc.tile_pool(name="big", bufs=2) as big,
            tc.tile_pool(name="small", bufs=2) as small,
        ):
            # counter output: cnt = j + i (int32), trivial
            jt = small.tile([P, T_LOC // P], I32, tag="jt")
            it = small.tile([P, T_LOC // P], I32, tag="it")
            ct = small.tile([P, T_LOC // P], I32, tag="ct")
            nc.sync.dma_start(jt[:], ji_d[:].rearrange("(p f) -> p f", p=P))
            nc.sync.dma_start(it[:], ii_d[:].rearrange("(p f) -> p f", p=P))
            nc.vector.tensor_add(ct[:], jt[:], it[:])
            nc.sync.dma_start(cnt_d[:].rearrange("(p f) -> p f", p=P), ct[:])

            for tt in range(N_TTILE):
                t0 = tt * P
                # per-t-tile tables
                dct = small.tile([P, TRACE], F32, tag="dct")
                c1t = small.tile([P, 2 * CTX], F32, tag="c1t")
                c2t = small.tile([P, 2 * CTX], F32, tag="c2t")
                nc.sync.dma_start(dct[:], dec_d[t0:t0 + P, :])
                nc.sync.dma_start(c1t[:], c1_d[t0:t0 + P, :])
                nc.sync.dma_start(c2t[:], c2_d[t0:t0 + P, :])

                c1b = (
                    c1t[:].rearrange("p (c two) -> p c two", two=2)
                    .unsqueeze(1).broadcast_to([P, RB, CTX, 2])
                )
                c2b = (
                    c2t[:].rearrange("p (c two) -> p c two", two=2)
                    .unsqueeze(1).broadcast_to([P, RB, CTX, 2])
                )

                for rb in range(N_RB):
                    f0 = rb * FD
                    sr = inp.tile([P, FD], F32, tag="sr")
                    si = inp.tile([P, FD], F32, tag="si")
                    xt = inp.tile([P, 2 * FD], F32, tag="xt")
                    nc.sync.dma_start(sr[:], sr_d[t0:t0 + P, f0:f0 + FD])
                    nc.sync.dma_start(si[:], si_d[t0:t0 + P, f0:f0 + FD])
                    nc.sync.dma_start(xt[:], x_d[t0:t0 + P, 2 * f0:2 * f0 + 2 * FD])

                    at = big.tile([P, 2 * FD], F32, tag="at")
                    bt = big.tile([P, 2 * FD], F32, tag="bt")

                    # ACT: in-place decay prescale, one [P,64] slice per r
                    for rl in range(RB):
                        rg = rb * RB + rl
                        s = slice(rl * CTX, (rl + 1) * CTX)
                        nc.scalar.activation(
                            sr[:, s], sr[:, s], CP, scale=dct[:, rg:rg + 1])
                        nc.scalar.activation(
                            si[:, s], si[:, s], CP, scale=dct[:, rg:rg + 1])

                    sr4 = (
                        sr[:].rearrange("p (r c) -> p r c", c=CTX)
                        .unsqueeze(3).broadcast_to([P, RB, CTX, 2])
                    )
                    si4 = (
                        si[:].rearrange("p (r c) -> p r c", c=CTX)
                        .unsqueeze(3).broadcast_to([P, RB, CTX, 2])
                    )
                    a4 = at[:].rearrange("p (r c two) -> p r c two", c=CTX, two=2)
                    b4 = bt[:].rearrange("p (r c two) -> p r c two", c=CTX, two=2)

                    # A = (sr*dec) x (cos,sin);  B = (si*dec) x (-sin,cos)
                    nc.vector.tensor_mul(a4, sr4, c1b)
                    nc.vector.tensor_mul(b4, si4, c2b)
                    # AB = rotation result, interleaved; out = AB + x
                    nc.vector.tensor_add(at[:], at[:], bt[:])
                    nc.vector.tensor_add(xt[:], at[:], xt[:])

                    nc.sync.dma_start(
                        out_d[t0:t0 + P, 2 * f0:2 * f0 + 2 * FD], xt[:]
                    )
    if cap_waits:
        cap_sync_waits(nc)
    return nc


_NC_CACHE: list[bass.Bass | None] = [None]


def _factor_tables(a, b, j):
    """decay/cos/sin tables [T, 64] computed with jnp on the default jax
    backend, mirroring the reference's expressions exactly (same ops, same
    shapes) so the table values match the reference bit-for-bit on
    whichever platform the reference runs."""
    import jax.numpy as jnp
    aj, bj, jj = jnp.asarray(a), jnp.asarray(b), jnp.asarray(j)
    t = jj.astype(jnp.float32)[:, None, None]
    decay = jnp.exp(-jnp.abs(aj)[None, :, None] * t)    # [T,TRACE,1]
    theta = bj[None, None, :] * t                       # [T,1,CTX]
    cosv = jnp.cos(theta)
    sinv = jnp.sin(theta)
    return (np.asarray(decay[:, :, 0]), np.asarray(cosv[:, 0, :]),
            np.asarray(sinv[:, 0, :]))


def make_in_maps(inputs):
    state_re, state_im = inputs["state_re"], inputs["state_im"]
    x_re, x_im = inputs["x_re"], inputs["x_im"]
    a, b, i, j = inputs["a"], inputs["b"], inputs["i"], inputs["j"]
    dec, cosv, sinv = _factor_tables(a, b, j)
    assert dec.dtype == np.float32 and cosv.dtype == np.float32

    csn1 = np.empty((T, 2 * CTX), dtype=np.float32)
    csn1[:, 0::2] = cosv
    csn1[:, 1::2] = sinv
    csn2 = np.empty((T, 2 * CTX), dtype=np.float32)
    csn2[:, 0::2] = -sinv
    csn2[:, 1::2] = cosv

    sr = np.ascontiguousarray(state_re.reshape(T, F))
    si = np.ascontiguousarray(state_im.reshape(T, F))
    x_int = np.stack(
        [x_re.reshape(T, F), x_im.reshape(T, F)], axis=-1
    ).reshape(T, 2 * F)

    in_maps = []
    for k in range(N_CORES):
        s = slice(k * T_LOC, (k + 1) * T_LOC)
        in_maps.append({
            "state_re": sr[s], "state_im": si[s],
            "x_int": np.ascontiguousarray(x_int[s]),
            "dec": np.ascontiguousarray(dec[s]),
            "csn1": np.ascontiguousarray(csn1[s]),
            "csn2": np.ascontiguousarray(csn2[s]),
            "jv": np.ascontiguousarray(j[s]),
            "iv": np.ascontiguousarray(i[s]),
        })
    return in_maps


def kernel(state_re, state_im, x_re, x_im, a, b, i, j):
    in_maps = make_in_maps(dict(
        state_re=state_re, state_im=state_im, x_re=x_re, x_im=x_im,
        a=a, b=b, i=i, j=j,
    ))

    if _NC_CACHE[0] is None:
        _NC_CACHE[0] = build_program()
    nc = _NC_CACHE[0]

    res = run_bass_kernel_spmd(nc, in_maps, core_ids=list(range(N_CORES)))
    out = np.concatenate(
        [r["out"].reshape(T_LOC, TRACE, CTX, 2) for r in res.results], axis=0
    )
    cnt = np.concatenate([r["cnt"] for r in res.results], axis=0).astype(np.int32)
    return out, cnt


# revision 19
# speedup vs baseline: 1.2977x; 1.1113x over previous
"""FFM cell (complex gated multiply-add) kernel for Trainium2, 8 NeuronCores.

Computes, for t in [0,T), r in [0,TRACE), c in [0,CTX):
    decay[t,r] = exp(-|a[r]| * t)        (t = float(j[t]))
    g_re[t,r,c] = decay[t,r] * cos(b[c]*t)
    g_im[t,r,c] = decay[t,r] * sin(b[c]*t)
    new_re = state_re*g_re - state_im*g_im + x_re
    new_im = state_re*g_im + state_im*g_re + x_im
Returns (stack([new_re,new_im], -1), j + i).

Strategy: shard T across the 8 cores (pure data parallel). Host precomputes
the small per-axis tables decay[T,64], cos[T,64], sin[T,64] with jnp on the
default jax backend (mirrors the reference's own table math exactly), plus
two interleaved trig tables csn1=(cos,sin) and csn2=(-sin,cos), and
interleaves x into x_int[T,r,c,2] (pure layout, no arithmetic).

On device, per [128t x 32r] tile:
  - ACT prescales state by decay in place: 64 small activation(Copy,
    scale=dec[:,r]) ops (per-partition scale = exact fp32 FMA). ACT has its
    own SBUF port, so this is truly additive capacity.
  - DVE does only 4 unit-stride f32 TT ops of 4096: A = srd (dup'd via a
    stride-0 4D AP) * csn1, B = sid * csn2, AB = A+B, out = AB + x_int.
    The output lands directly in (re,im)-interleaved layout - no strided
    writes, no GpSimd (whose SBUF port is shared with DVE and measurably
    degrades it).
DMA: 48.4 MiB per core total at ~358 GB/s -> ~141 us/core roofline.
"""

import numpy as np

import concourse.bass as bass
import concourse.mybir as mybir
import concourse.tile as tile
from concourse.bass_utils import run_bass_kernel_spmd

T, TRACE, CTX = 4096, 64, 64
N_CORES = 8
T_LOC = T // N_CORES            # 512 t-rows per core
F = TRACE * CTX                 # 4096 features per t
P = 128                         # SBUF partitions (t per tile)
RB = 32                         # trace-block per iteration
FD = RB * CTX                   # 2048 state elements per iteration
N_TTILE = T_LOC // P            # 4 t-tiles per core
N_RB = TRACE // RB              # 2 r-blocks per t-tile

F32 = mybir.dt.float32
I32 = mybir.dt.int32

MAX_WAITS_PER_INST = 1


def cap_sync_waits(nc: bass.Bass, limit: int = MAX_WAITS_PER_INST):
    """walrus's CoreV3 codegen rejects instructions carrying more than one
    sync wait ("Too many sync wait commands"). Hoist extra waits onto
    single-wait NOPs inserted just before, on the same engine."""
    for bb in nc.main_func.blocks:
        insts = bb.instructions
        idx = 0
        while idx < len(insts):
            ins = insts[idx]
            si = ins.sync_info
            if si is not None and si.on_wait and len(si.on_wait) > limit:
                waits = list(si.on_wait)
                si.on_wait = waits[:limit]
                extra = waits[limit:]
                for w in extra:
                    nop = mybir.InstNoOp(
                        name=nc.get_next_instruction_name(),
                        engine=ins.engine,
                        sync_info=mybir.SyncInfo(on_wait=[w], on_update=[]),
                        bass_nofuse=True,
                    )
                    insts.insert(idx, nop)
                    idx += 1
            idx += 1


class SplitDrainTileContext(tile.TileContext):
    """Tile's kernel-tail drain piles every outstanding sem wait onto one
    Drain instruction; walrus rejects >1 sync waits per instruction. Spread
    the waits over standalone single-wait NOPs instead."""

    def _drain_and_barrier(self, tick_clock, wait_clock):
        nc = self.nc
        probe = nc.sync.drain()
        wait_clock.add_sem_waits(
            probe.ins, tile.ScopedClock({None: tick_clock.global_clock})
        )
        si = probe.ins.sync_info
        waits = list(si.on_wait or []) if si else []
        if si and si.on_wait and len(waits) > 1:
            si.on_wait = [waits[0]]
            for w in waits[1:]:
                nop = nc.sync.nop(nofuse=True)
                nsi = nop.ins.sync_info
                if nsi is None:
                    nop.ins.sync_info = mybir.SyncInfo(on_wait=[w], on_update=[])
                else:
                    nsi.on_wait = (nsi.on_wait or []) + [w]
        nc.all_engine_barrier()
        assert self.sems is not None
        popped = nc._tile_sem_poison_stack.pop()
        assert popped is self._sem_poison
        nc.clear_and_free_semaphores(list(self.sems.allocated().values()))
        nc.all_engine_barrier()


def build_program(cap_waits: bool = True) -> bass.Bass:
    nc = bass.Bass()

    sr_d = nc.declare_dram_parameter("state_re", [T_LOC, F], F32, isOutput=False)
    si_d = nc.declare_dram_parameter("state_im", [T_LOC, F], F32, isOutput=False)
    x_d = nc.declare_dram_parameter("x_int", [T_LOC, 2 * F], F32, isOutput=False)
    dec_d = nc.declare_dram_parameter("dec", [T_LOC, TRACE], F32, isOutput=False)
    c1_d = nc.declare_dram_parameter("csn1", [T_LOC, 2 * CTX], F32, isOutput=False)
    c2_d = nc.declare_dram_parameter("csn2", [T_LOC, 2 * CTX], F32, isOutput=False)
    ji_d = nc.declare_dram_parameter("jv", [T_LOC], I32, isOutput=False)
    ii_d = nc.declare_dram_parameter("iv", [T_LOC], I32, isOutput=False)

    out_d = nc.declare_dram_parameter("out", [T_LOC, 2 * F], F32, isOutput=True)
    cnt_d = nc.declare_dram_parameter("cnt", [T_LOC], I32, isOutput=True)

    CP = mybir.ActivationFunctionType.Copy

    with SplitDrainTileContext(nc) as tc:
        with (
            tc.tile_pool(name="inp", bufs=3) as inp,
            tc.tile_pool(name="big", bufs=2) as big,
            tc.tile_pool(name="small", bufs=2) as small,
        ):
            # counter output: cnt = j + i (int32), trivial
            jt = small.tile([P, T_LOC // P], I32, tag="jt")
            it = small.tile([P, T_LOC // P], I32, tag="it")
            ct = small.tile([P, T_LOC // P], I32, tag="ct")
            nc.sync.dma_start(jt[:], ji_d[:].rearrange("(p f) -> p f", p=P))
            nc.sync.dma_start(it[:], ii_d[:].rearrange("(p f) -> p f", p=P))
            nc.vector.tensor_add(ct[:], jt[:], it[:])
            nc.sync.dma_start(cnt_d[:].rearrange("(p f) -> p f", p=P), ct[:])

            for tt in range(N_TTILE):
                t0 = tt * P
                # per-t-tile tables
                dct = small.tile([P, TRACE], F32, tag="dct")
                c1t = small.tile([P, 2 * CTX], F32, tag="c1t")
                c2t = small.tile([P, 2 * CTX], F32, tag="c2t")
                nc.sync.dma_start(dct[:], dec_d[t0:t0 + P, :])
                nc.sync.dma_start(c1t[:], c1_d[t0:t0 + P, :])
                nc.sync.dma_start(c2t[:], c2_d[t0:t0 + P, :])

                c1b = (
                    c1t[:].rearrange("p (c two) -> p c two", two=2)
                    .unsqueeze(1).broadcast_to([P, RB, CTX, 2])
                )
                c2b = (
                    c2t[:].rearrange("p (c two) -> p c two", two=2)
                    .unsqueeze(1).broadcast_to([P, RB, CTX, 2])
                )

                for rb in range(N_RB):
                    f0 = rb * FD
                    # combined state tile: [sr | si] halves
                    st = inp.tile([P, 2 * FD], F32, tag="st")
                    xt = inp.tile([P, 2 * FD], F32, tag="xt")
                    nc.sync.dma_start(st[:, 0:FD], sr_d[t0:t0 + P, f0:f0 + FD])
                    nc.sync.dma_start(st[:, FD:2 * FD], si_d[t0:t0 + P, f0:f0 + FD])
                    nc.sync.dma_start(xt[:], x_d[t0:t0 + P, 2 * f0:2 * f0 + 2 * FD])

                    at = big.tile([P, 2 * FD], F32, tag="at")
                    bt = big.tile([P, 2 * FD], F32, tag="bt")

                    # ACT: in-place decay prescale; one op per r covers the
                    # matching [P,64] slice of BOTH halves via a 2D free AP
                    st3 = st[:].rearrange("p (h r c) -> p h r c", h=2, c=CTX)
                    for rl in range(RB):
                        rg = rb * RB + rl
                        sl = st3[:, :, rl, :]
                        nc.scalar.activation(sl, sl, CP, scale=dct[:, rg:rg + 1])

                    sr4 = (
                        st[:, 0:FD].rearrange("p (r c) -> p r c", c=CTX)
                        .unsqueeze(3).broadcast_to([P, RB, CTX, 2])
                    )
                    si4 = (
                        st[:, FD:2 * FD].rearrange("p (r c) -> p r c", c=CTX)
                        .unsqueeze(3).broadcast_to([P, RB, CTX, 2])
                    )
                    a4 = at[:].rearrange("p (r c two) -> p r c two", c=CTX, two=2)
                    b4 = bt[:].rearrange("p (r c two) -> p r c two", c=CTX, two=2)

                    # A = (sr*dec) x (cos,sin);  B = (si*dec) x (-sin,cos)
                    nc.vector.tensor_mul(a4, sr4, c1b)
                    nc.vector.tensor_mul(b4, si4, c2b)
                    # AB = rotation result, interleaved; out = AB + x
                    nc.vector.tensor_add(at[:], at[:], bt[:])
                    nc.vector.tensor_add(xt[:], at[:], xt[:])

                    nc.sync.dma_start(
                        out_d[t0:t0 + P, 2 * f0:2 * f0 + 2 * FD], xt[:]
                    )
    if cap_waits:
        cap_sync_waits(nc)
    return nc


_NC_CACHE: list[bass.Bass | None] = [None]


def _factor_tables(a, b, j):
    """decay/cos/sin tables [T, 64] computed with jnp on the default jax
    backend, mirroring the reference's expressions exactly (same ops, same
    shapes) so the table values match the reference bit-for-bit on
    whichever platform the reference runs."""
    import jax.numpy as jnp
    aj, bj, jj = jnp.asarray(a), jnp.asarray(b), jnp.asarray(j)
    t = jj.astype(jnp.float32)[:, None, None]
    decay = jnp.exp(-jnp.abs(aj)[None, :, None] * t)    # [T,TRACE,1]
    theta = bj[None, None, :] * t                       # [T,1,CTX]
    cosv = jnp.cos(theta)
    sinv = jnp.sin(theta)
    return (np.asarray(decay[:, :, 0]), np.asarray(cosv[:, 0, :]),
            np.asarray(sinv[:, 0, :]))


def make_in_maps(inputs):
    state_re, state_im = inputs["state_re"], inputs["state_im"]
    x_re, x_im = inputs["x_re"], inputs["x_im"]
    a, b, i, j = inputs["a"], inputs["b"], inputs["i"], inputs["j"]
    dec, cosv, sinv = _factor_tables(a, b, j)
    assert dec.dtype == np.float32 and cosv.dtype == np.float32

    csn1 = np.empty((T, 2 * CTX), dtype=np.float32)
    csn1[:, 0::2] = cosv
    csn1[:, 1::2] = sinv
    csn2 = np.empty((T, 2 * CTX), dtype=np.float32)
    csn2[:, 0::2] = -sinv
    csn2[:, 1::2] = cosv

    sr = np.ascontiguousarray(state_re.reshape(T, F))
    si = np.ascontiguousarray(state_im.reshape(T, F))
    x_int = np.stack(
        [x_re.reshape(T, F), x_im.reshape(T, F)], axis=-1
    ).reshape(T, 2 * F)

    in_maps = []
    for k in range(N_CORES):
        s = slice(k * T_LOC, (k + 1) * T_LOC)
        in_maps.append({
            "state_re": sr[s], "state_im": si[s],
            "x_int": np.ascontiguousarray(x_int[s]),
            "dec": np.ascontiguousarray(dec[s]),
            "csn1": np.ascontiguousarray(csn1[s]),
            "csn2": np.ascontiguousarray(csn2[s]),
            "jv": np.ascontiguousarray(j[s]),
            "iv": np.ascontiguousarray(i[s]),
        })
    return in_maps


def kernel(state_re, state_im, x_re, x_im, a, b, i, j):
    in_maps = make_in_maps(dict(
        state_re=state_re, state_im=state_im, x_re=x_re, x_im=x_im,
        a=a, b=b, i=i, j=j,
    ))

    if _NC_CACHE[0] is None:
        _NC_CACHE[0] = build_program()
    nc = _NC_CACHE[0]

    res = run_bass_kernel_spmd(nc, in_maps, core_ids=list(range(N_CORES)))
    out = np.concatenate(
        [r["out"].reshape(T_LOC, TRACE, CTX, 2) for r in res.results], axis=0
    )
    cnt = np.concatenate([r["cnt"] for r in res.results], axis=0).astype(np.int32)
    return out, cnt
